# revision 17
# baseline (speedup 1.0000x reference)
"""AdaptiveAttention (B=2, S=2048, HID=2048, NH=16, HD=128) on 8 TRN2 cores.

Tensor-parallel over heads (2 heads/core).  v3 — PE-row economy (each
matmul costs moving-cols + 128 weight-load rows on the PE array):
  * q/k/v projections run chunk-PAIRED: one stationary load per (ko, head)
    covers two 512-row chunks (the halves of a 2-bank PSUM tile).
  * v is projected TRANSPOSED like q/k (same pairing economics, ~60% fewer
    PE rows than natural-layout chains) and flipped back to natural
    [keys, hd] layout with 128x128 DMA-XBAR transposes (off-engine).
  * gate: x row-sums via one DVE tensor_reduce per chunk, emitted BEFORE
    the chunk's matmuls so the DVE drains during PE-heavy projection;
    the Wg matmuls then run at N=1.
  * softmax denominators: exp tiles are accumulated into a bf16 SBUF
    accumulator by the DVE (2x packed mode); one ones-matmul per
    (q-tile, head) replicates the cross-partition total.
  * attention emits score matmuls, a paired 1024-wide exp, and the
    previous pair's PV matmuls round-robin; each group's sums/rec/norm
    tail is deferred into the next group so the PE never waits on the
    DVE.
  * AllGathers run per (batch, q-tile); o_proj chains are rc-paired
    except the final half (shorter tail); o_proj(b0) overlaps attn-phase
    emission of batch 1.  Large loads are split 4-ways across DMA queues;
    x/weights are host-pretiled so every DMA line is >=8KB.
Matmul datapath bf16, fp32 PSUM; rope tables and exp inputs fp32.
"""
import os
import sys
import types

import numpy as np

if "/opt/trn_rl_repo" not in sys.path:
    sys.path.insert(0, "/opt/trn_rl_repo")

B, S, HID = 2, 2048, 2048
NH, HD = 16, 128
ROPE_BASE = 10000.0
NC = 8                    # cores
HPC = NH // NC            # heads per core (2)
HDC = HPC * HD            # head dims per core (256)
ROWS = B * S
KO = HID // 128           # 16 contraction tiles
CH = 512                  # projection row-chunk
QT = 512                  # attention q tile
NCH = S // CH             # chunks per batch (4)
INV_SQRT_HD = 1.0 / float(np.sqrt(HD))

_CACHE = {}


def _install_ntff_hook():
    """Best-effort: register the NTFF profile hook bass_utils expects under
    axon (the image's antenv lacks axon_hooks), so trace=True works."""
    try:
        import antenv  # noqa: F401
        if "antenv.axon_hooks" in sys.modules:
            return
        mod = types.ModuleType("antenv.axon_hooks")
        _state = {"hook": None}
        mod.set_axon_ntff_profile_hook = lambda h: _state.__setitem__("hook", h)
        mod.get_axon_ntff_profile_hook = lambda: _state["hook"]
        sys.modules["antenv.axon_hooks"] = mod
        from trn_agent_boot.trn_boot import _ntff_profile_via_ctypes
        so = "/opt/axon/libaxon_pjrt.so"
        if os.path.exists(so):
            hook = _ntff_profile_via_ctypes(so)
            if hook is not None:
                mod.set_axon_ntff_profile_hook(hook)
    except Exception:
        pass


def _build():
    import concourse.mybir as mybir
    import concourse.tile as tile
    from concourse import bacc

    f32 = mybir.dt.float32
    bf16 = mybir.dt.bfloat16
    AF = mybir.ActivationFunctionType
    MUL = mybir.AluOpType.mult
    ADD = mybir.AluOpType.add
    AXX = mybir.AxisListType.X

    nc = bacc.Bacc("TRN2", target_bir_lowering=False, debug=False, num_devices=NC)

    def din(name, shape, dt=bf16):
        return nc.dram_tensor(name, shape, dt, kind="ExternalInput").ap()

    # host-pretiled inputs: partition dim second, per-partition data contiguous
    xt_t = din("xt_t", [B * NCH, 128, KO, CH])     # x chunks, transposed+tiled
    wq_t = din("wq_t", [128, KO, HDC])
    wk_t = din("wk_t", [128, KO, HDC])
    wv_t = din("wv_t", [128, KO, HDC])
    wo_t = din("wo_t", [128, KO, HDC])
    wg_t = din("wg_t", [128, KO, HPC])
    bg = din("bg", [HPC, 1], f32)
    cosT = din("cosT", [HD, ROWS], f32)            # rope tables, [d, b*S+s]
    sinT = din("sinT", [HD, ROWS], f32)            # rotate-half sign folded in
    tri = din("tri", [128, 128])                   # tri[kk,t] = 1.0 if t >= kk
    ones = din("ones", [128, 128])
    out = nc.dram_tensor("out", [HDC, ROWS], f32, kind="ExternalOutput").ap()

    with tile.TileContext(nc) as tc:
        with tc.tile_pool(name="constp", bufs=1) as constp, \
             tc.tile_pool(name="wpool", bufs=1) as wpool, \
             tc.tile_pool(name="bpool", bufs=1) as bpool, \
             tc.tile_pool(name="stream", bufs=4) as stream, \
             tc.tile_pool(name="gtp", bufs=2) as gtp, \
             tc.tile_pool(name="work", bufs=3) as work, \
             tc.tile_pool(name="espool", bufs=4) as espool, \
             tc.tile_pool(name="saccp", bufs=2) as saccp, \
             tc.tile_pool(name="small", bufs=2) as small, \
             tc.tile_pool(name="psW", bufs=3, space="PSUM") as psW, \
             tc.tile_pool(name="psB", bufs=2, space="PSUM") as psB, \
             tc.tile_pool(name="dram", bufs=1, space="DRAM") as dram:

            wq_sb = wpool.tile([128, KO, HDC], bf16)
            wk_sb = wpool.tile([128, KO, HDC], bf16)
            wv_sb = wpool.tile([128, KO, HDC], bf16)
            wo_sb = wpool.tile([128, KO, HDC], bf16)
            wg_sb = wpool.tile([128, KO, HPC], bf16)
            tri_sb = constp.tile([128, 128], bf16)
            ones_sb = constp.tile([128, 128], bf16)
            bg_sb = constp.tile([HPC, 1], f32)
            _loaded = set()

            def lazy(sb_t, src, key, split=4):
                # quarter big loads across DMA queues
                if key in _loaded:
                    return
                _loaded.add(key)
                if split == 1:
                    nc.sync.dma_start(sb_t, src)
                    return
                ksz = KO // split
                for q in range(split):
                    nc.sync.dma_start(sb_t[:, q * ksz:(q + 1) * ksz],
                                      src[:, q * ksz:(q + 1) * ksz])

            def load_xt_pair(b, pair):
                xts = []
                for ch in pair:
                    xt = stream.tile([128, KO, CH], bf16, tag="stream",
                                     name="xt")
                    for q in range(4):
                        nc.sync.dma_start(xt[:, q * 4:(q + 1) * 4],
                                          xt_t[b * NCH + ch][:, q * 4:(q + 1) * 4])
                    xts.append(xt)
                return xts

            # per-batch persistent tiles (tags reused across batches)
            def batch_tiles(b):
                cos_sb = bpool.tile([HD, S], f32, tag="cos", name="cos_sb")
                sin_sb = bpool.tile([HD, S], f32, tag="sin", name="sin_sb")
                qfin = [bpool.tile([128, HPC, CH], bf16, tag=f"qfin{c}",
                                   name=f"qfin{c}") for c in range(NCH)]
                kfin = [bpool.tile([128, HPC, CH], bf16, tag=f"kfin{c}",
                                   name=f"kfin{c}") for c in range(NCH)]
                vsb = [bpool.tile([128, CH // 128, HDC], bf16, tag=f"vsb{c}",
                                  name=f"vsb{c}") for c in range(NCH)]
                gxs = bpool.tile([128, KO, NCH], bf16, tag="gxs", name="gxs")
                gbc = bpool.tile([128, HPC], f32, tag="gbc", name="gbc")
                return cos_sb, sin_sb, qfin, kfin, vsb, gxs, gbc

            # ---------------- projections for one batch ----------------
            def proj_batch(b, bt, pre=None):
                cos_sb, sin_sb, qfin, kfin, vsb, gxs, gbc = bt
                lazy(wq_sb, wq_t, "wq")
                for pi, pair in enumerate(((0, 1), (2, 3))):
                    if pi == 0 and pre is not None:
                        xts = pre
                    else:
                        xts = load_xt_pair(b, pair)
                    if pi == 0:
                        lazy(wk_sb, wk_t, "wk")
                        lazy(wv_sb, wv_t, "wv")
                        nc.sync.dma_start(cos_sb, cosT[:, b * S:(b + 1) * S])
                        nc.sync.dma_start(sin_sb, sinT[:, b * S:(b + 1) * S])
                    # gate row-sum of first chunk before PE work: DVE drains
                    # while the PE runs the sweeps
                    with nc.allow_low_precision(
                            reason="x row-sum in bf16; final dot is fp32"):
                        nc.vector.tensor_reduce(
                            gxs[:, :, pair[0]:pair[0] + 1], xts[0], AXX, ADD)
                    # q/k, chunk-paired chains
                    for (w_sb, fin) in ((wq_sb, qfin), (wk_sb, kfin)):
                        for hh in range(HPC):
                            psp = psW.tile([128, 2 * QT], f32, tag="w",
                                           name="psp")
                            for ko in range(KO):
                                for i in range(2):
                                    nc.tensor.matmul(
                                        psp[:, i * QT:(i + 1) * QT],
                                        lhsT=w_sb[:, ko,
                                                  hh * 128:(hh + 1) * 128],
                                        rhs=xts[i][:, ko],
                                        start=(ko == 0), stop=(ko == KO - 1))
                            for i, ch in enumerate(pair):
                                ps = psp[:, i * QT:(i + 1) * QT]
                                c0 = ch * CH
                                raw = work.tile([128, CH], bf16, tag="raw",
                                                name="raw")
                                nc.scalar.activation(raw, ps, AF.Copy)
                                rsh = work.tile([128, CH], bf16, tag="rsh",
                                                name="rsh")
                                nc.sync.dma_start(rsh[0:64, :], raw[64:128, :])
                                nc.sync.dma_start(rsh[64:128, :], raw[0:64, :])
                                dst = fin[ch][:, hh, :]
                                nc.vector.tensor_mul(dst, ps,
                                                     cos_sb[:, c0:c0 + CH])
                                tmp = work.tile([128, CH], f32, tag="ropetmp",
                                                name="tmp")
                                nc.vector.tensor_mul(tmp, rsh,
                                                     sin_sb[:, c0:c0 + CH])
                                nc.vector.tensor_add(dst, dst, tmp)
                    with nc.allow_low_precision(
                            reason="x row-sum in bf16; final dot is fp32"):
                        nc.vector.tensor_reduce(
                            gxs[:, :, pair[1]:pair[1] + 1], xts[1], AXX, ADD)
                    # v: transposed sweeps (paired like q/k), then DMA-XBAR
                    # 128x128 transposes back to natural [keys, hd] layout
                    for hh in range(HPC):
                        psp = psW.tile([128, 2 * QT], f32, tag="w", name="psp")
                        for ko in range(KO):
                            for i in range(2):
                                nc.tensor.matmul(
                                    psp[:, i * QT:(i + 1) * QT],
                                    lhsT=wv_sb[:, ko, hh * 128:(hh + 1) * 128],
                                    rhs=xts[i][:, ko],
                                    start=(ko == 0), stop=(ko == KO - 1))
                        for i, ch in enumerate(pair):
                            vt = work.tile([128, CH], bf16, tag="vt",
                                           name="vt")
                            nc.scalar.activation(
                                vt, psp[:, i * QT:(i + 1) * QT], AF.Copy)
                            for rt in range(CH // 128):
                                nc.sync.dma_start(
                                    vsb[ch][:, rt, hh * 128:(hh + 1) * 128],
                                    vt[:, rt * 128:(rt + 1) * 128],
                                    transpose=True)

            # gates = sigmoid(mean_rows(x) @ WgT + bg), broadcast to 128 parts
            def gates_batch(b, bt):
                cos_sb, sin_sb, qfin, kfin, vsb, gxs, gbc = bt
                lazy(wg_sb, wg_t, "wg", split=1)
                lazy(bg_sb, bg, "bg", split=1)
                gsum = small.tile([128, KO, 1], bf16, tag="gsum", name="gsum")
                with nc.allow_low_precision(
                        reason="sum of 4 chunk sums; final dot in fp32 PSUM"):
                    nc.vector.tensor_reduce(gsum, gxs, AXX, ADD)
                psg = psB.tile([128, QT], f32, tag="pv", name="psg")
                for ko in range(KO):
                    nc.tensor.matmul(psg[0:HPC, 0:1], lhsT=wg_sb[:, ko],
                                     rhs=gsum[:, ko],
                                     start=(ko == 0), stop=(ko == KO - 1))
                gates = small.tile([HPC, 1], f32, tag="gates", name="gates")
                nc.scalar.activation(gates, psg[0:HPC, 0:1], AF.Sigmoid,
                                     bias=bg_sb, scale=1.0 / S)
                gdr = dram.tile([HPC, 1], f32, name=f"gdr{b}", tag=f"gdr{b}")
                nc.sync.dma_start(gdr, gates)
                nc.sync.dma_start(
                    gbc, gdr[:].rearrange("p o -> o p").to_broadcast((128, HPC)))

            # ---------------- attention for one batch ----------------
            ag_out = [[None] * NCH for _ in range(B)]

            def attn_batch(b, bt):
                cos_sb, sin_sb, qfin, kfin, vsb, gxs, gbc = bt
                lazy(tri_sb, tri, "tri", split=1)
                lazy(ones_sb, ones, "ones", split=1)
                ag_in = [dram.tile([HDC, QT], bf16, name=f"agin{b}_{q}",
                                   tag=f"agin{b}_{q}") for q in range(NCH)]
                for q in range(NCH):
                    ag_out[b][q] = dram.tile(
                        [NH * HD, QT], bf16, addr_space="Shared",
                        name=f"agout{b}_{q}", tag=f"agout{b}_{q}")
                # finalize (sums/rec/norm/gather) for a finished group is
                # emitted one group later, after the next group's first
                # scores, so the PE never waits on the DVE add chain
                pending = []

                def flush_pending():
                    while pending:
                        pending.pop(0)()

                for qt in range(S // QT):
                    q0 = qt * QT
                    qch = q0 // CH
                    # key tiles: (kt, col0, n); diagonal group gets col0 > 0
                    tiles = []
                    for kt in range((qt + 1) * (QT // 128)):
                        m = kt - qt * (QT // 128)
                        col0 = 128 * m if m > 0 else 0
                        tiles.append((kt, col0, QT - col0, m >= 0))
                    pairs = [tuple(tiles[i:i + 2])
                             for i in range(0, len(tiles), 2)]
                    for h in range(HPC):
                        pso = psB.tile([128, QT], f32, tag="pv", name="pso")
                        sacc = saccp.tile([128, QT], bf16, tag="sacc",
                                          name="sacc")
                        epairs = []
                        for j, pr in enumerate(pairs):
                            # scores for this pair into one 2-bank PSUM tile
                            psc = psW.tile([128, 2 * QT], f32, tag="w",
                                           name="psc")
                            offs, width = [], 0
                            for (kt, col0, n, diag) in pr:
                                nc.tensor.matmul(
                                    psc[:, width:width + n],
                                    lhsT=kfin[kt // 4][:, h, (kt % 4) * 128:
                                                       (kt % 4) * 128 + 128],
                                    rhs=qfin[qch][:, h, col0:QT],
                                    start=True, stop=True)
                                offs.append(width)
                                width += n
                            if j == 1:
                                flush_pending()
                            es = espool.tile([128, 2 * QT], bf16, tag="es",
                                             name="es")
                            nc.scalar.activation(es[:, :width], psc[:, :width],
                                                 AF.Exp, scale=INV_SQRT_HD)
                            for (kt, col0, n, diag), off in zip(pr, offs):
                                if diag:
                                    nc.vector.tensor_mul(
                                        es[:, off:off + 128],
                                        es[:, off:off + 128], tri_sb)
                                if j == 0 and off == 0:
                                    nc.vector.tensor_copy(
                                        out=sacc[:, col0:],
                                        in_=es[:, off:off + n])
                                else:
                                    nc.vector.tensor_add(
                                        sacc[:, col0:], sacc[:, col0:],
                                        es[:, off:off + n])
                            epairs.append((pr, offs, es))
                            if j >= 1:
                                emit_pv(pso, epairs[j - 1], h, vsb,
                                        first=(j == 1), last=False)
                        emit_pv(pso, epairs[-1], h, vsb,
                                first=(len(pairs) == 1), last=True)

                        def finalize(pso=pso, sacc=sacc, qt=qt, h=h):
                            # one ones-matmul replicates column sums to all
                            # partitions
                            pss = psW.tile([128, 2 * QT], f32, tag="w",
                                           name="pss")[:, :QT]
                            nc.tensor.matmul(pss, lhsT=ones_sb, rhs=sacc,
                                             start=True, stop=True)
                            rec = work.tile([128, QT], f32, tag="rec",
                                            name="rec")
                            nc.vector.reciprocal_approx_fast(rec, pss)
                            ot = work.tile([128, QT], bf16, tag="ot",
                                           name="ot")
                            nc.vector.scalar_tensor_tensor(
                                ot, pso, gbc[:, h:h + 1], rec,
                                op0=MUL, op1=MUL)
                            nc.sync.dma_start(
                                ag_in[qt][h * 128:(h + 1) * 128, :], ot)
                            if h == HPC - 1:
                                nc.gpsimd.collective_compute(
                                    "AllGather", mybir.AluOpType.bypass,
                                    replica_groups=[list(range(NC))],
                                    ins=[ag_in[qt][:].opt()],
                                    outs=[ag_out[b][qt][:].opt()])
                        pending.append(finalize)
                flush_pending()

            def emit_pv(pso, epair, h, vsb, first, last):
                pr, offs, es = epair
                npr = len(pr)
                for idx, ((kt, col0, n, diag), off) in enumerate(zip(pr, offs)):
                    nc.tensor.matmul(
                        pso[:, col0:],
                        lhsT=vsb[kt // 4][:, kt % 4, h * 128:(h + 1) * 128],
                        rhs=es[:, off:off + n],
                        start=(first and idx == 0),
                        stop=(last and idx == npr - 1))

            # ---------------- o_proj for one batch ----------------
            def oproj_batch(b, pairing):
                lazy(wo_sb, wo_t, "wo")
                for rcs in pairing:
                    gts = []
                    for rc in rcs:
                        gt = gtp.tile([128, KO, QT], bf16, tag="gt", name="gt")
                        ag3 = ag_out[b][rc][:].rearrange(
                            "(ko p) r -> p ko r", p=128)
                        for kq in range(4):
                            nc.sync.dma_start(gt[:, kq * 4:(kq + 1) * 4],
                                              ag3[:, kq * 4:(kq + 1) * 4])
                        gts.append(gt)
                    for ct in range(HDC // 128):
                        pso2 = psW.tile([128, 2 * QT], f32, tag="w",
                                        name="pso2")
                        for ko in range(KO):
                            for i in range(len(rcs)):
                                nc.tensor.matmul(
                                    pso2[:, i * QT:(i + 1) * QT],
                                    lhsT=wo_sb[:, ko, ct * 128:(ct + 1) * 128],
                                    rhs=gts[i][:, ko],
                                    start=(ko == 0), stop=(ko == KO - 1))
                        for i, rc in enumerate(rcs):
                            oc_sb = work.tile([128, QT], f32, tag="oc",
                                              name="oc_sb", bufs=2)
                            nc.scalar.activation(
                                oc_sb, pso2[:, i * QT:(i + 1) * QT], AF.Copy)
                            nc.sync.dma_start(
                                out[ct * 128:(ct + 1) * 128,
                                    b * S + rc * QT:b * S + (rc + 1) * QT],
                                oc_sb)

            # ================= schedule =================
            bt0 = batch_tiles(0)
            proj_batch(0, bt0)
            gates_batch(0, bt0)
            pre1 = load_xt_pair(1, (0, 1))     # prefetch during attn(b0)
            attn_batch(0, bt0)
            bt1 = batch_tiles(1)
            proj_batch(1, bt1, pre=pre1)
            gates_batch(1, bt1)
            oproj_batch(0, [(0, 1), (2, 3)])
            attn_batch(1, bt1)
            oproj_batch(1, [(0, 1), (2,), (3,)])
    nc.compile()
    return nc


def _prepare_in_maps(hidden_states, position_ids, Wq, Wk, Wv, Wo, Wg, bg):
    import ml_dtypes
    b16 = ml_dtypes.bfloat16

    x = np.ascontiguousarray(hidden_states.reshape(ROWS, HID), dtype=np.float32)
    # [chunks, 128, KO, CH]: per-(chunk, partition) data contiguous
    xt_t = np.ascontiguousarray(
        x.reshape(B * NCH, CH, KO, 128).transpose(0, 3, 2, 1)).astype(b16)

    def tile_w(WT):  # [HID, cols] -> [128, KO, cols]
        return np.ascontiguousarray(
            WT.reshape(KO, 128, WT.shape[1]).transpose(1, 0, 2)).astype(b16)

    WqT = Wq.T.astype(np.float32)
    WkT = Wk.T.astype(np.float32)
    WvT = Wv.T.astype(np.float32)
    WoT = Wo.T.astype(np.float32)
    WgT = Wg.T.astype(np.float32)

    inv_freq = 1.0 / (ROPE_BASE ** (np.arange(0, HD, 2, dtype=np.float32) / HD))
    freqs = np.arange(S, dtype=np.float32)[:, None] * inv_freq[None, :]
    emb = np.concatenate([freqs, freqs], axis=-1)          # [S, HD]
    cos_t = np.cos(emb).astype(np.float32)
    sin_t = np.sin(emb).astype(np.float32)
    pos = np.asarray(position_ids).astype(np.int64)
    cosT = np.ascontiguousarray(
        np.concatenate([cos_t[pos[b]] for b in range(B)], axis=0).T)
    sinT = np.ascontiguousarray(
        np.concatenate([sin_t[pos[b]] for b in range(B)], axis=0).T)
    sinT[:HD // 2] *= -1.0   # rotate-half sign folded into the table

    tri = (np.arange(128)[None, :] >= np.arange(128)[:, None]).astype(b16)
    ones = np.ones((128, 128), dtype=b16)
    bgc = np.asarray(bg, dtype=np.float32)

    in_maps = []
    for c in range(NC):
        s0 = c * HDC
        in_maps.append({
            "xt_t": xt_t,
            "wq_t": tile_w(np.ascontiguousarray(WqT[:, s0:s0 + HDC])),
            "wk_t": tile_w(np.ascontiguousarray(WkT[:, s0:s0 + HDC])),
            "wv_t": tile_w(np.ascontiguousarray(WvT[:, s0:s0 + HDC])),
            "wo_t": tile_w(np.ascontiguousarray(WoT[:, s0:s0 + HDC])),
            "wg_t": tile_w(np.ascontiguousarray(
                WgT[:, c * HPC:(c + 1) * HPC])),
            "bg": np.ascontiguousarray(bgc[c * HPC:(c + 1) * HPC, None]),
            "cosT": cosT, "sinT": sinT,
            "tri": tri, "ones": ones,
        })
    return in_maps


LAST_RESULT = None


def kernel(hidden_states, attention_mask, position_ids, Wq, Wk, Wv, Wo, Wg, bg):
    global LAST_RESULT
    _install_ntff_hook()
    from concourse.bass_utils import run_bass_kernel_spmd

    if "nc" not in _CACHE:
        _CACHE["nc"] = _build()
    nc = _CACHE["nc"]

    in_maps = _prepare_in_maps(hidden_states, position_ids, Wq, Wk, Wv, Wo, Wg, bg)
    res = run_bass_kernel_spmd(nc, in_maps, core_ids=list(range(NC)))
    LAST_RESULT = res
    blocks = [res.results[c]["out"] for c in range(NC)]     # each [HDC, ROWS]
    full_T = np.concatenate(blocks, axis=0)                 # [HID, ROWS]
    return np.ascontiguousarray(full_T.T).reshape(B, S, HID).astype(np.float32)


# revision 24
# speedup vs baseline: 1.0294x; 1.0294x over previous
"""AdaptiveAttention (B=2, S=2048, HID=2048, NH=16, HD=128) on 8 TRN2 cores.

Tensor-parallel over heads (2 heads/core).  v3 — PE-row economy (each
matmul costs moving-cols + 128 weight-load rows on the PE array):
  * q/k/v projections run chunk-PAIRED: one stationary load per (ko, head)
    covers two 512-row chunks (the halves of a 2-bank PSUM tile).
  * v is projected TRANSPOSED like q/k (same pairing economics, ~60% fewer
    PE rows than natural-layout chains) and flipped back to natural
    [keys, hd] layout with 128x128 DMA-XBAR transposes (off-engine).
  * gate: x row-sums via one DVE tensor_reduce per chunk, emitted BEFORE
    the chunk's matmuls so the DVE drains during PE-heavy projection;
    the Wg matmuls then run at N=1.
  * softmax denominators: exp tiles are accumulated into a bf16 SBUF
    accumulator by the DVE (2x packed mode); one ones-matmul per
    (q-tile, head) replicates the cross-partition total.
  * attention emits score matmuls, a paired 1024-wide exp, and the
    previous pair's PV matmuls round-robin; each group's sums/rec/norm
    tail is deferred into the next group so the PE never waits on the
    DVE.
  * AllGathers run per (batch, q-tile); o_proj chains are rc-paired
    except the final half (shorter tail); o_proj(b0) overlaps attn-phase
    emission of batch 1.  Large loads are split 4-ways across DMA queues;
    x/weights are host-pretiled so every DMA line is >=8KB.
Matmul datapath bf16, fp32 PSUM; rope tables and exp inputs fp32.
"""
import os
import sys
import types

import numpy as np

if "/opt/trn_rl_repo" not in sys.path:
    sys.path.insert(0, "/opt/trn_rl_repo")

B, S, HID = 2, 2048, 2048
NH, HD = 16, 128
ROPE_BASE = 10000.0
NC = 8                    # cores
HPC = NH // NC            # heads per core (2)
HDC = HPC * HD            # head dims per core (256)
ROWS = B * S
KO = HID // 128           # 16 contraction tiles
CH = 512                  # projection row-chunk
QT = 512                  # attention q tile
NCH = S // CH             # chunks per batch (4)
INV_SQRT_HD = 1.0 / float(np.sqrt(HD))

_CACHE = {}


def _install_ntff_hook():
    """Best-effort: register the NTFF profile hook bass_utils expects under
    axon (the image's antenv lacks axon_hooks), so trace=True works."""
    try:
        import antenv  # noqa: F401
        if "antenv.axon_hooks" in sys.modules:
            return
        mod = types.ModuleType("antenv.axon_hooks")
        _state = {"hook": None}
        mod.set_axon_ntff_profile_hook = lambda h: _state.__setitem__("hook", h)
        mod.get_axon_ntff_profile_hook = lambda: _state["hook"]
        sys.modules["antenv.axon_hooks"] = mod
        from trn_agent_boot.trn_boot import _ntff_profile_via_ctypes
        so = "/opt/axon/libaxon_pjrt.so"
        if os.path.exists(so):
            hook = _ntff_profile_via_ctypes(so)
            if hook is not None:
                mod.set_axon_ntff_profile_hook(hook)
    except Exception:
        pass


def _build():
    import concourse.mybir as mybir
    import concourse.tile as tile
    from concourse import bacc

    f32 = mybir.dt.float32
    bf16 = mybir.dt.bfloat16
    AF = mybir.ActivationFunctionType
    MUL = mybir.AluOpType.mult
    ADD = mybir.AluOpType.add
    AXX = mybir.AxisListType.X

    nc = bacc.Bacc("TRN2", target_bir_lowering=False, debug=False, num_devices=NC)

    def din(name, shape, dt=bf16):
        return nc.dram_tensor(name, shape, dt, kind="ExternalInput").ap()

    # host-pretiled inputs: partition dim second, per-partition data contiguous
    xt_t = din("xt_t", [B * NCH, 128, KO, CH])     # x chunks, transposed+tiled
    wq_t = din("wq_t", [128, KO, HDC])
    wk_t = din("wk_t", [128, KO, HDC])
    wv_t = din("wv_t", [128, KO, HDC])
    wo_t = din("wo_t", [128, KO, HDC])
    wg_t = din("wg_t", [128, KO, HPC])
    bg = din("bg", [HPC, 1], f32)
    cosT = din("cosT", [HD, ROWS], f32)            # rope tables, [d, b*S+s]
    sinT = din("sinT", [HD, ROWS], f32)            # rotate-half sign folded in
    tri = din("tri", [128, 128])                   # tri[kk,t] = 1.0 if t >= kk
    ones = din("ones", [128, 128])
    ident = din("ident", [128, 128])               # PE-transpose identity
    out = nc.dram_tensor("out", [HDC, ROWS], f32, kind="ExternalOutput").ap()

    with tile.TileContext(nc) as tc:
        with tc.tile_pool(name="constp", bufs=1) as constp, \
             tc.tile_pool(name="wpool", bufs=1) as wpool, \
             tc.tile_pool(name="bpool", bufs=1) as bpool, \
             tc.tile_pool(name="stream", bufs=4) as stream, \
             tc.tile_pool(name="gtp", bufs=2) as gtp, \
             tc.tile_pool(name="work", bufs=3) as work, \
             tc.tile_pool(name="espool", bufs=4) as espool, \
             tc.tile_pool(name="saccp", bufs=2) as saccp, \
             tc.tile_pool(name="small", bufs=2) as small, \
             tc.tile_pool(name="psW", bufs=3, space="PSUM") as psW, \
             tc.tile_pool(name="psB", bufs=2, space="PSUM") as psB, \
             tc.tile_pool(name="dram", bufs=1, space="DRAM") as dram:

            wq_sb = wpool.tile([128, KO, HDC], bf16)
            wk_sb = wpool.tile([128, KO, HDC], bf16)
            wv_sb = wpool.tile([128, KO, HDC], bf16)
            wo_sb = wpool.tile([128, KO, HDC], bf16)
            wg_sb = wpool.tile([128, KO, HPC], bf16)
            tri_sb = constp.tile([128, 128], bf16)
            ones_sb = constp.tile([128, 128], bf16)
            ident_sb = constp.tile([128, 128], bf16)
            bg_sb = constp.tile([HPC, 1], f32)
            _loaded = set()

            def lazy(sb_t, src, key, split=4):
                # quarter big loads across DMA queues
                if key in _loaded:
                    return
                _loaded.add(key)
                if split == 1:
                    nc.sync.dma_start(sb_t, src)
                    return
                ksz = KO // split
                for q in range(split):
                    nc.sync.dma_start(sb_t[:, q * ksz:(q + 1) * ksz],
                                      src[:, q * ksz:(q + 1) * ksz])

            def load_xt_pair(b, pair):
                xts = []
                for ch in pair:
                    xt = stream.tile([128, KO, CH], bf16, tag="stream",
                                     name="xt")
                    for q in range(4):
                        nc.sync.dma_start(xt[:, q * 4:(q + 1) * 4],
                                          xt_t[b * NCH + ch][:, q * 4:(q + 1) * 4])
                    xts.append(xt)
                return xts

            # per-batch persistent tiles (tags reused across batches)
            def batch_tiles(b):
                cos_sb = bpool.tile([HD, S], f32, tag="cos", name="cos_sb")
                sin_sb = bpool.tile([HD, S], f32, tag="sin", name="sin_sb")
                qfin = [bpool.tile([128, HPC, CH], bf16, tag=f"qfin{c}",
                                   name=f"qfin{c}") for c in range(NCH)]
                kfin = [bpool.tile([128, HPC, CH], bf16, tag=f"kfin{c}",
                                   name=f"kfin{c}") for c in range(NCH)]
                vsb = [bpool.tile([128, CH // 128, HDC], bf16, tag=f"vsb{c}",
                                  name=f"vsb{c}") for c in range(NCH)]
                gxs = bpool.tile([128, KO, NCH], bf16, tag="gxs", name="gxs")
                gbc = bpool.tile([128, HPC], f32, tag="gbc", name="gbc")
                return cos_sb, sin_sb, qfin, kfin, vsb, gxs, gbc

            # ---------------- projections for one batch ----------------
            def proj_batch(b, bt, pre=None):
                cos_sb, sin_sb, qfin, kfin, vsb, gxs, gbc = bt
                lazy(wq_sb, wq_t, "wq")
                for pi, pair in enumerate(((0, 1), (2, 3))):
                    if pi == 0 and pre is not None:
                        xts = pre
                    else:
                        xts = load_xt_pair(b, pair)
                    if pi == 0:
                        lazy(wk_sb, wk_t, "wk")
                        lazy(wv_sb, wv_t, "wv")
                        nc.sync.dma_start(cos_sb, cosT[:, b * S:(b + 1) * S])
                        nc.sync.dma_start(sin_sb, sinT[:, b * S:(b + 1) * S])
                    # gate row-sum of first chunk before PE work: DVE drains
                    # while the PE runs the sweeps
                    with nc.allow_low_precision(
                            reason="x row-sum in bf16; final dot is fp32"):
                        nc.vector.tensor_reduce(
                            gxs[:, :, pair[0]:pair[0] + 1], xts[0], AXX, ADD)
                    # q/k, chunk-paired chains
                    for (w_sb, fin) in ((wq_sb, qfin), (wk_sb, kfin)):
                        for hh in range(HPC):
                            psp = psW.tile([128, 2 * QT], f32, tag="w",
                                           name="psp")
                            for ko in range(KO):
                                for i in range(2):
                                    nc.tensor.matmul(
                                        psp[:, i * QT:(i + 1) * QT],
                                        lhsT=w_sb[:, ko,
                                                  hh * 128:(hh + 1) * 128],
                                        rhs=xts[i][:, ko],
                                        start=(ko == 0), stop=(ko == KO - 1))
                            for i, ch in enumerate(pair):
                                ps = psp[:, i * QT:(i + 1) * QT]
                                c0 = ch * CH
                                raw = work.tile([128, CH], bf16, tag="raw",
                                                name="raw")
                                nc.scalar.activation(raw, ps, AF.Copy)
                                rsh = work.tile([128, CH], bf16, tag="rsh",
                                                name="rsh")
                                # partition swap on the idle GpSimd SWDGE
                                # queue; keeps the SP DGE sequencer free
                                nc.gpsimd.dma_start(rsh[0:64, :],
                                                    raw[64:128, :])
                                nc.gpsimd.dma_start(rsh[64:128, :],
                                                    raw[0:64, :])
                                dst = fin[ch][:, hh, :]
                                nc.vector.tensor_mul(dst, ps,
                                                     cos_sb[:, c0:c0 + CH])
                                tmp = work.tile([128, CH], f32, tag="ropetmp",
                                                name="tmp")
                                nc.vector.tensor_mul(tmp, rsh,
                                                     sin_sb[:, c0:c0 + CH])
                                nc.vector.tensor_add(dst, dst, tmp)
                    with nc.allow_low_precision(
                            reason="x row-sum in bf16; final dot is fp32"):
                        nc.vector.tensor_reduce(
                            gxs[:, :, pair[1]:pair[1] + 1], xts[1], AXX, ADD)
                    # v: transposed sweeps (paired like q/k), flipped back to
                    # natural [keys, hd] layout with 128x128 PE transposes
                    lazy(ident_sb, ident, "ident", split=1)
                    for hh in range(HPC):
                        psp = psW.tile([128, 2 * QT], f32, tag="w", name="psp")
                        for ko in range(KO):
                            for i in range(2):
                                nc.tensor.matmul(
                                    psp[:, i * QT:(i + 1) * QT],
                                    lhsT=wv_sb[:, ko, hh * 128:(hh + 1) * 128],
                                    rhs=xts[i][:, ko],
                                    start=(ko == 0), stop=(ko == KO - 1))
                        for i, ch in enumerate(pair):
                            vt = work.tile([128, CH], bf16, tag="vt",
                                           name="vt")
                            nc.scalar.activation(
                                vt, psp[:, i * QT:(i + 1) * QT], AF.Copy)
                            psT = psB.tile([128, QT], bf16, tag="pv",
                                           name="psT")
                            for rt in range(CH // 128):
                                nc.tensor.transpose(
                                    psT[:, rt * 128:(rt + 1) * 128],
                                    vt[:, rt * 128:(rt + 1) * 128], ident_sb)
                            nc.scalar.activation(
                                vsb[ch][:, :, hh * 128:(hh + 1) * 128],
                                psT[:].rearrange("p (rt c) -> p rt c", rt=4),
                                AF.Copy)

            # gates = sigmoid(mean_rows(x) @ WgT + bg), broadcast to 128 parts
            def gates_batch(b, bt):
                cos_sb, sin_sb, qfin, kfin, vsb, gxs, gbc = bt
                lazy(wg_sb, wg_t, "wg", split=1)
                lazy(bg_sb, bg, "bg", split=1)
                gsum = small.tile([128, KO, 1], bf16, tag="gsum", name="gsum")
                with nc.allow_low_precision(
                        reason="sum of 4 chunk sums; final dot in fp32 PSUM"):
                    nc.vector.tensor_reduce(gsum, gxs, AXX, ADD)
                psg = psB.tile([128, QT], f32, tag="pv", name="psg")
                for ko in range(KO):
                    nc.tensor.matmul(psg[0:HPC, 0:1], lhsT=wg_sb[:, ko],
                                     rhs=gsum[:, ko],
                                     start=(ko == 0), stop=(ko == KO - 1))
                gates = small.tile([HPC, 1], f32, tag="gates", name="gates")
                nc.scalar.activation(gates, psg[0:HPC, 0:1], AF.Sigmoid,
                                     bias=bg_sb, scale=1.0 / S)
                gdr = dram.tile([HPC, 1], f32, name=f"gdr{b}", tag=f"gdr{b}")
                nc.sync.dma_start(gdr, gates)
                nc.sync.dma_start(
                    gbc, gdr[:].rearrange("p o -> o p").to_broadcast((128, HPC)))

            # ---------------- attention for one batch ----------------
            ag_out = [[None] * NCH for _ in range(B)]

            def attn_batch(b, bt):
                cos_sb, sin_sb, qfin, kfin, vsb, gxs, gbc = bt
                lazy(tri_sb, tri, "tri", split=1)
                lazy(ones_sb, ones, "ones", split=1)
                ag_in = [dram.tile([HDC, QT], bf16, name=f"agin{b}_{q}",
                                   tag=f"agin{b}_{q}") for q in range(NCH)]
                for q in range(NCH):
                    ag_out[b][q] = dram.tile(
                        [NH * HD, QT], bf16, addr_space="Shared",
                        name=f"agout{b}_{q}", tag=f"agout{b}_{q}")
                # finalize (sums/rec/norm/gather) for a finished group is
                # emitted one group later, after the next group's first
                # scores, so the PE never waits on the DVE add chain
                pending = []

                def flush_pending():
                    while pending:
                        pending.pop(0)()

                for qt in range(S // QT):
                    q0 = qt * QT
                    qch = q0 // CH
                    # key tiles: (kt, col0, n); diagonal group gets col0 > 0
                    tiles = []
                    for kt in range((qt + 1) * (QT // 128)):
                        m = kt - qt * (QT // 128)
                        col0 = 128 * m if m > 0 else 0
                        tiles.append((kt, col0, QT - col0, m >= 0))
                    pairs = [tuple(tiles[i:i + 2])
                             for i in range(0, len(tiles), 2)]
                    for h in range(HPC):
                        pso = psB.tile([128, QT], f32, tag="pv", name="pso")
                        sacc = saccp.tile([128, QT], bf16, tag="sacc",
                                          name="sacc")
                        epairs = []
                        for j, pr in enumerate(pairs):
                            # scores for this pair into one 2-bank PSUM tile
                            psc = psW.tile([128, 2 * QT], f32, tag="w",
                                           name="psc")
                            offs, width = [], 0
                            for (kt, col0, n, diag) in pr:
                                nc.tensor.matmul(
                                    psc[:, width:width + n],
                                    lhsT=kfin[kt // 4][:, h, (kt % 4) * 128:
                                                       (kt % 4) * 128 + 128],
                                    rhs=qfin[qch][:, h, col0:QT],
                                    start=True, stop=True)
                                offs.append(width)
                                width += n
                            if j == 1:
                                flush_pending()
                            es = espool.tile([128, 2 * QT], bf16, tag="es",
                                             name="es")
                            nc.scalar.activation(es[:, :width], psc[:, :width],
                                                 AF.Exp, scale=INV_SQRT_HD)
                            for (kt, col0, n, diag), off in zip(pr, offs):
                                if diag:
                                    nc.vector.tensor_mul(
                                        es[:, off:off + 128],
                                        es[:, off:off + 128], tri_sb)
                                if j == 0 and off == 0:
                                    nc.vector.tensor_copy(
                                        out=sacc[:, col0:],
                                        in_=es[:, off:off + n])
                                else:
                                    nc.vector.tensor_add(
                                        sacc[:, col0:], sacc[:, col0:],
                                        es[:, off:off + n])
                            epairs.append((pr, offs, es))
                            if j >= 1:
                                emit_pv(pso, epairs[j - 1], h, vsb,
                                        first=(j == 1), last=False)
                        emit_pv(pso, epairs[-1], h, vsb,
                                first=(len(pairs) == 1), last=True)

                        def finalize(pso=pso, sacc=sacc, qt=qt, h=h):
                            # one ones-matmul replicates column sums to all
                            # partitions
                            pss = psW.tile([128, 2 * QT], f32, tag="w",
                                           name="pss")[:, :QT]
                            nc.tensor.matmul(pss, lhsT=ones_sb, rhs=sacc,
                                             start=True, stop=True)
                            rec = work.tile([128, QT], f32, tag="rec",
                                            name="rec")
                            nc.vector.reciprocal_approx_fast(rec, pss)
                            ot = work.tile([128, QT], bf16, tag="ot",
                                           name="ot")
                            nc.vector.scalar_tensor_tensor(
                                ot, pso, gbc[:, h:h + 1], rec,
                                op0=MUL, op1=MUL)
                            nc.sync.dma_start(
                                ag_in[qt][h * 128:(h + 1) * 128, :], ot)
                            if h == HPC - 1:
                                nc.gpsimd.collective_compute(
                                    "AllGather", mybir.AluOpType.bypass,
                                    replica_groups=[list(range(NC))],
                                    ins=[ag_in[qt][:].opt()],
                                    outs=[ag_out[b][qt][:].opt()])
                        pending.append(finalize)
                flush_pending()

            def emit_pv(pso, epair, h, vsb, first, last):
                pr, offs, es = epair
                npr = len(pr)
                for idx, ((kt, col0, n, diag), off) in enumerate(zip(pr, offs)):
                    nc.tensor.matmul(
                        pso[:, col0:],
                        lhsT=vsb[kt // 4][:, kt % 4, h * 128:(h + 1) * 128],
                        rhs=es[:, off:off + n],
                        start=(first and idx == 0),
                        stop=(last and idx == npr - 1))

            # ---------------- o_proj for one batch ----------------
            def oproj_batch(b, pairing):
                lazy(wo_sb, wo_t, "wo")
                for rcs in pairing:
                    gts = []
                    for rc in rcs:
                        gt = gtp.tile([128, KO, QT], bf16, tag="gt", name="gt")
                        ag3 = ag_out[b][rc][:].rearrange(
                            "(ko p) r -> p ko r", p=128)
                        # bulk loads on the ACT DGE queue (idle in o_proj)
                        for kq in range(2):
                            nc.scalar.dma_start(gt[:, kq * 8:(kq + 1) * 8],
                                                ag3[:, kq * 8:(kq + 1) * 8])
                        gts.append(gt)
                    for ct in range(HDC // 128):
                        pso2 = psW.tile([128, 2 * QT], f32, tag="w",
                                        name="pso2")
                        for ko in range(KO):
                            for i in range(len(rcs)):
                                nc.tensor.matmul(
                                    pso2[:, i * QT:(i + 1) * QT],
                                    lhsT=wo_sb[:, ko, ct * 128:(ct + 1) * 128],
                                    rhs=gts[i][:, ko],
                                    start=(ko == 0), stop=(ko == KO - 1))
                        for i, rc in enumerate(rcs):
                            oc_sb = work.tile([128, QT], f32, tag="oc",
                                              name="oc_sb", bufs=2)
                            nc.scalar.activation(
                                oc_sb, pso2[:, i * QT:(i + 1) * QT], AF.Copy)
                            nc.sync.dma_start(
                                out[ct * 128:(ct + 1) * 128,
                                    b * S + rc * QT:b * S + (rc + 1) * QT],
                                oc_sb)

            # ================= schedule =================
            bt0 = batch_tiles(0)
            proj_batch(0, bt0)
            gates_batch(0, bt0)
            pre1 = load_xt_pair(1, (0, 1))     # prefetch during attn(b0)
            attn_batch(0, bt0)
            bt1 = batch_tiles(1)
            proj_batch(1, bt1, pre=pre1)
            gates_batch(1, bt1)
            oproj_batch(0, [(0, 1), (2, 3)])
            attn_batch(1, bt1)
            oproj_batch(1, [(0, 1), (2,), (3,)])
    nc.compile()
    return nc


def _prepare_in_maps(hidden_states, position_ids, Wq, Wk, Wv, Wo, Wg, bg):
    import ml_dtypes
    b16 = ml_dtypes.bfloat16

    x = np.ascontiguousarray(hidden_states.reshape(ROWS, HID), dtype=np.float32)
    # [chunks, 128, KO, CH]: per-(chunk, partition) data contiguous
    xt_t = np.ascontiguousarray(
        x.reshape(B * NCH, CH, KO, 128).transpose(0, 3, 2, 1)).astype(b16)

    def tile_w(WT):  # [HID, cols] -> [128, KO, cols]
        return np.ascontiguousarray(
            WT.reshape(KO, 128, WT.shape[1]).transpose(1, 0, 2)).astype(b16)

    WqT = Wq.T.astype(np.float32)
    WkT = Wk.T.astype(np.float32)
    WvT = Wv.T.astype(np.float32)
    WoT = Wo.T.astype(np.float32)
    WgT = Wg.T.astype(np.float32)

    inv_freq = 1.0 / (ROPE_BASE ** (np.arange(0, HD, 2, dtype=np.float32) / HD))
    freqs = np.arange(S, dtype=np.float32)[:, None] * inv_freq[None, :]
    emb = np.concatenate([freqs, freqs], axis=-1)          # [S, HD]
    cos_t = np.cos(emb).astype(np.float32)
    sin_t = np.sin(emb).astype(np.float32)
    pos = np.asarray(position_ids).astype(np.int64)
    cosT = np.ascontiguousarray(
        np.concatenate([cos_t[pos[b]] for b in range(B)], axis=0).T)
    sinT = np.ascontiguousarray(
        np.concatenate([sin_t[pos[b]] for b in range(B)], axis=0).T)
    sinT[:HD // 2] *= -1.0   # rotate-half sign folded into the table

    tri = (np.arange(128)[None, :] >= np.arange(128)[:, None]).astype(b16)
    ones = np.ones((128, 128), dtype=b16)
    ident = np.eye(128, dtype=b16)
    bgc = np.asarray(bg, dtype=np.float32)

    in_maps = []
    for c in range(NC):
        s0 = c * HDC
        in_maps.append({
            "xt_t": xt_t,
            "wq_t": tile_w(np.ascontiguousarray(WqT[:, s0:s0 + HDC])),
            "wk_t": tile_w(np.ascontiguousarray(WkT[:, s0:s0 + HDC])),
            "wv_t": tile_w(np.ascontiguousarray(WvT[:, s0:s0 + HDC])),
            "wo_t": tile_w(np.ascontiguousarray(WoT[:, s0:s0 + HDC])),
            "wg_t": tile_w(np.ascontiguousarray(
                WgT[:, c * HPC:(c + 1) * HPC])),
            "bg": np.ascontiguousarray(bgc[c * HPC:(c + 1) * HPC, None]),
            "cosT": cosT, "sinT": sinT,
            "tri": tri, "ones": ones, "ident": ident,
        })
    return in_maps


LAST_RESULT = None


def kernel(hidden_states, attention_mask, position_ids, Wq, Wk, Wv, Wo, Wg, bg):
    global LAST_RESULT
    _install_ntff_hook()
    from concourse.bass_utils import run_bass_kernel_spmd

    if "nc" not in _CACHE:
        _CACHE["nc"] = _build()
    nc = _CACHE["nc"]

    in_maps = _prepare_in_maps(hidden_states, position_ids, Wq, Wk, Wv, Wo, Wg, bg)
    res = run_bass_kernel_spmd(nc, in_maps, core_ids=list(range(NC)))
    LAST_RESULT = res
    blocks = [res.results[c]["out"] for c in range(NC)]     # each [HDC, ROWS]
    full_T = np.concatenate(blocks, axis=0)                 # [HID, ROWS]
    return np.ascontiguousarray(full_T.T).reshape(B, S, HID).astype(np.float32)


# revision 35
# speedup vs baseline: 1.1951x; 1.1609x over previous
"""AdaptiveAttention (B=2, S=2048, HID=2048, NH=16, HD=128) on 8 TRN2 cores.

Strategy: tensor-parallel over heads (2 heads/core).  All device matmuls
run with the contraction dim on the partition axis, so the host wrapper
pre-transposes x and the weights.  Attention runs in transposed layout:
  scoresT[keys, q] = kT.T @ qT    (k-tile stationary, q moving, N=512)
  expS = exp(scoresT / sqrt(HD))  (causal: fully-masked key tiles skipped,
                                   diagonal 128x128 masked via a 0/1 tile)
  outT[hd, q]  = v.T @ expS       (accumulated over key tiles)
  sums[128, q] = ones128.T @ expS (softmax denominator replicated on all
                                   partitions -> normalization is pure DVE)
  outT *= gate/sums
Per q-tile all scores/exp are issued first, then the PV/sums chains run
back-to-back so the PE never waits on the scalar engine.  q/k run
transposed at N=512; v is projected in natural [rows, hd] layout.  RoPE is
applied as qfin = q*cos + rot(q)*sin where rot is a pure 64-partition
rotation (two partition-offset SBUF copies; the rotate-half sign is
folded into the host-side sin table), costing the PE nothing.
Per-head outputs are AllGathered per batch in two sequence halves
(rank-major concat = head-dim order); both o_proj passes run last so
the collectives overlap compute.  Matmul datapath is bf16 (FWL weight loads,
fp32 PSUM accumulation); rope tables and the exp input stay fp32.
"""
import os
import sys
import types

import numpy as np

if "/opt/trn_rl_repo" not in sys.path:
    sys.path.insert(0, "/opt/trn_rl_repo")

B, S, HID = 2, 2048, 2048
NH, HD = 16, 128
ROPE_BASE = 10000.0
NC = 8                    # cores
HPC = NH // NC            # heads per core
HDC = HPC * HD            # head dims per core (256)
ROWS = B * S
KO = HID // 128           # 16 contraction tiles
CH = 512                  # projection row-chunk
QT = 512                  # attention q tile
OC = 512                  # o_proj row chunk
NCH = S // CH             # chunks per batch (4)
INV_SQRT_HD = 1.0 / float(np.sqrt(HD))

_CACHE = {}


def _install_ntff_hook():
    """Best-effort: register the NTFF profile hook bass_utils expects under
    axon (the image's antenv lacks axon_hooks), so trace=True works."""
    try:
        import antenv  # noqa: F401
        if "antenv.axon_hooks" in sys.modules:
            return
        mod = types.ModuleType("antenv.axon_hooks")
        _state = {"hook": None}
        mod.set_axon_ntff_profile_hook = lambda h: _state.__setitem__("hook", h)
        mod.get_axon_ntff_profile_hook = lambda: _state["hook"]
        sys.modules["antenv.axon_hooks"] = mod
        from trn_agent_boot.trn_boot import _ntff_profile_via_ctypes
        so = "/opt/axon/libaxon_pjrt.so"
        if os.path.exists(so):
            hook = _ntff_profile_via_ctypes(so)
            if hook is not None:
                mod.set_axon_ntff_profile_hook(hook)
    except Exception:
        pass


def _build():
    import concourse.mybir as mybir
    import concourse.tile as tile
    from concourse import bacc

    f32 = mybir.dt.float32
    bf16 = mybir.dt.bfloat16
    AF = mybir.ActivationFunctionType
    MUL = mybir.AluOpType.mult

    nc = bacc.Bacc("TRN2", target_bir_lowering=False, debug=False, num_devices=NC)

    def din(name, shape, dt=bf16):
        return nc.dram_tensor(name, shape, dt, kind="ExternalInput").ap()

    # host-pretiled: partition dim second, per-partition data contiguous
    xt_t = din("xt_t", [B * NCH, 128, KO, CH])  # x chunks, transposed+tiled
    wq_t = din("wq_t", [128, KO, HDC])          # per-core head slice of Wq.T
    wk_t = din("wk_t", [128, KO, HDC])
    wv_t = din("wv_t", [128, KO, HDC])
    wo_t = din("wo_t", [128, KO, HDC])          # per-core col slice of Wo.T
    wg_t = din("wg_t", [128, KO, HPC])          # per-core cols of Wg.T
    bg = din("bg", [HPC, 1], f32)
    cosT = din("cosT", [HD, ROWS], f32)         # rope tables, [d, b*S+s]
    sinT = din("sinT", [HD, ROWS], f32)
    pmatT = din("pmatT", [HD, HD])              # rotate-half matrix P.T
    tri = din("tri", [128, 128])                # tri[kk,t] = 1.0 if t >= kk
    ones = din("ones", [128, 128])              # all-ones matrix
    ident = din("ident", [128, 128])            # identity (PE transpose)
    out = nc.dram_tensor("out", [HDC, ROWS], f32, kind="ExternalOutput").ap()

    with tile.TileContext(nc) as tc:
        with tc.tile_pool(name="const", bufs=1) as constp, \
             tc.tile_pool(name="wpool", bufs=1) as wpool, \
             tc.tile_pool(name="bpool", bufs=1) as bpool, \
             tc.tile_pool(name="stream", bufs=4) as stream, \
             tc.tile_pool(name="work", bufs=4) as work, \
             tc.tile_pool(name="espool", bufs=18) as espool, \
             tc.tile_pool(name="small", bufs=2) as small, \
             tc.tile_pool(name="psA", bufs=3, space="PSUM") as psA, \
             tc.tile_pool(name="psB", bufs=2, space="PSUM") as psB, \
             tc.tile_pool(name="psS", bufs=2, space="PSUM") as psS, \
             tc.tile_pool(name="psG", bufs=1, space="PSUM") as psG, \
             tc.tile_pool(name="dram", bufs=1, space="DRAM") as dram:

            # persistent tiles; DMAs are emitted lazily right before first use
            wq_sb = wpool.tile([128, KO, HDC], bf16)
            wk_sb = wpool.tile([128, KO, HDC], bf16)
            wv_sb = wpool.tile([128, KO, HDC], bf16)
            wo_sb = wpool.tile([128, KO, HDC], bf16)
            wg_sb = wpool.tile([128, KO, HPC], bf16)
            tri_sb = constp.tile([128, 128], bf16)
            ones_sb = constp.tile([128, 128], bf16)
            bg_sb = constp.tile([HPC, 1], f32)
            _loaded = set()

            def lazy(sb_t, src, key, split=4):
                # quarter big loads so they stripe across DMA queues
                if key in _loaded:
                    return
                _loaded.add(key)
                if split == 1:
                    nc.sync.dma_start(sb_t, src)
                    return
                ksz = KO // split
                for q in range(split):
                    nc.sync.dma_start(sb_t[:, q * ksz:(q + 1) * ksz],
                                      src[:, q * ksz:(q + 1) * ksz])

            lazy(wq_sb, wq_t, "wq")

            ag_outs = []
            for b in range(B):
                r0 = b * S
                cos_sb = bpool.tile([HD, S], f32, tag="cos")
                sin_sb = bpool.tile([HD, S], f32, tag="sin")
                # per-chunk tensors so attention can start before the whole
                # projection phase finishes (fine-grained tile deps)
                qfin = [bpool.tile([128, HPC, CH], bf16, tag=f"qfin{c}",
                                   name=f"qfin{c}") for c in range(NCH)]
                kfin = [bpool.tile([128, HPC, CH], bf16, tag=f"kfin{c}",
                                   name=f"kfin{c}") for c in range(NCH)]
                vsb = [bpool.tile([128, CH // 128, HDC], bf16, tag=f"vsb{c}",
                                  name=f"vsb{c}") for c in range(NCH)]
                gacc = bpool.tile([HPC, NCH], f32, tag="gacc")

                # ================= projections =================
                for ch in range(NCH):
                    c0 = ch * CH
                    xt = stream.tile([128, KO, CH], bf16, tag="stream")
                    for q in range(4):
                        nc.sync.dma_start(xt[:, q * 4:(q + 1) * 4],
                                          xt_t[b * NCH + ch][:, q * 4:(q + 1) * 4])
                    lazy(wk_sb, wk_t, "wk")
                    lazy(wv_sb, wv_t, "wv")
                    if ch == 0:
                        nc.sync.dma_start(cos_sb, cosT[:, r0:r0 + S])
                        nc.sync.dma_start(sin_sb, sinT[:, r0:r0 + S])
                    for (w_sb, fin) in ((wq_sb, qfin[ch]), (wk_sb, kfin[ch])):
                        for hh in range(HPC):
                            ps = psA.tile([128, QT], f32, tag="mm", name="ps_qk")
                            for ko in range(KO):
                                nc.tensor.matmul(
                                    ps, lhsT=w_sb[:, ko, hh * 128:(hh + 1) * 128],
                                    rhs=xt[:, ko],
                                    start=(ko == 0), stop=(ko == KO - 1))
                            raw = work.tile([128, CH], bf16, tag="raw")
                            nc.scalar.activation(raw, ps, AF.Copy)
                            rsh = work.tile([128, CH], bf16, tag="rsh")
                            nc.sync.dma_start(rsh[0:64, :], raw[64:128, :])
                            nc.sync.dma_start(rsh[64:128, :], raw[0:64, :])
                            dst = fin[:, hh, :]
                            nc.vector.tensor_mul(dst, ps, cos_sb[:, c0:c0 + CH])
                            tmp = work.tile([128, CH], f32, tag="ropetmp")
                            nc.vector.tensor_mul(tmp, rsh, sin_sb[:, c0:c0 + CH])
                            nc.vector.tensor_add(dst, fin[:, hh, :], tmp)
                    # v (natural layout)
                    for rt in range(CH // 128):
                        psv = psB.tile([128, QT], f32, tag="pv",
                                       name="psv")[:, :HDC]
                        for ko in range(KO):
                            nc.tensor.matmul(
                                psv, lhsT=xt[:, ko, rt * 128:(rt + 1) * 128],
                                rhs=wv_sb[:, ko],
                                start=(ko == 0), stop=(ko == KO - 1))
                        nc.scalar.activation(vsb[ch][:, rt], psv, AF.Copy)
                    # gate partial
                    lazy(wg_sb, wg_t, "wg", split=1)
                    psg = psG.tile([HPC, CH], f32, tag="pg")
                    for ko in range(KO):
                        nc.tensor.matmul(psg, lhsT=wg_sb[:, ko], rhs=xt[:, ko],
                                         start=(ko == 0), stop=(ko == KO - 1))
                    nc.vector.tensor_reduce(gacc[:, ch:ch + 1], psg,
                                            mybir.AxisListType.X,
                                            mybir.AluOpType.add)

                # gates = sigmoid(mean @ WgT + bg), broadcast to 128 partitions
                lazy(bg_sb, bg, "bg", split=1)
                glin = small.tile([HPC, 1], f32, tag="glin")
                nc.vector.tensor_reduce(glin, gacc, mybir.AxisListType.X,
                                        mybir.AluOpType.add)
                gates = small.tile([HPC, 1], f32, tag="gates")
                nc.scalar.activation(gates, glin, AF.Sigmoid,
                                     bias=bg_sb, scale=1.0 / S)
                gdr = dram.tile([HPC, 1], f32, name=f"gdr{b}", tag=f"gdr{b}")
                nc.sync.dma_start(gdr, gates)
                gbc = bpool.tile([128, HPC], f32, tag="gbc")
                nc.sync.dma_start(
                    gbc, gdr[:].rearrange("p o -> o p").to_broadcast((128, HPC)))

                # ================= attention =================
                lazy(tri_sb, tri, "tri", split=1)
                lazy(ones_sb, ones, "ones", split=1)
                ag_in = [dram.tile([HDC, S // 2], bf16, name=f"agin{b}_{i}",
                                   tag=f"agin{b}_{i}") for i in range(2)]
                ag_out = [dram.tile([NH * HD, S // 2], bf16, addr_space="Shared",
                                    name=f"agout{b}_{i}", tag=f"agout{b}_{i}")
                          for i in range(2)]
                qt_order = range(S // QT)
                done_halves = set()
                for qt in qt_order:
                    q0 = qt * QT
                    kmax = (qt + 1) * (QT // 128)
                    qch, qoff = q0 // CH, q0 % CH
                    for h in range(HPC):
                        pso = psB.tile([128, QT], f32, tag="pv", name="pso")
                        pss = psS.tile([128, QT], f32, tag="sums")
                        ess = []

                        def emit_pv_sums(i):
                            # PV + sums steps for tile i, interleaved into
                            # the score stream so the PE fills the slots
                            # where it would otherwise wait on exp
                            kt, col0, es = ess[i]
                            nc.tensor.matmul(
                                pso[:, col0:],
                                lhsT=vsb[kt // 4][:, kt % 4,
                                                  h * 128:(h + 1) * 128],
                                rhs=es, start=(i == 0), stop=(i == kmax - 1))
                            nc.tensor.matmul(
                                pss[:, col0:], lhsT=ones_sb, rhs=es,
                                start=(i == 0), stop=(i == kmax - 1))

                        for kt in range(kmax):
                            m = kt - qt * (QT // 128)   # >=0 on diagonal tiles
                            col0 = 128 * m if m > 0 else 0
                            n = QT - col0
                            psc = psA.tile([128, QT], f32, tag="mm",
                                           name="psc")[:, :n]
                            nc.tensor.matmul(
                                psc,
                                lhsT=kfin[kt // 4][:, h, (kt % 4) * 128:
                                                   (kt % 4) * 128 + 128],
                                rhs=qfin[qch][:, h, qoff + col0:qoff + QT],
                                start=True, stop=True)
                            es = espool.tile([128, QT], bf16, tag="es",
                                             name="es")[:, :n]
                            nc.scalar.activation(es, psc, AF.Exp,
                                                 scale=INV_SQRT_HD)
                            if m >= 0:
                                nc.vector.tensor_mul(es[:, :128], es[:, :128],
                                                     tri_sb)
                            ess.append((kt, col0, es))
                            if kt >= 2:
                                emit_pv_sums(kt - 2)
                        emit_pv_sums(kmax - 2)
                        emit_pv_sums(kmax - 1)
                        # normalize: outT *= gate / sums (sums replicated on
                        # all 128 partitions by the all-ones stationary)
                        rec = work.tile([128, QT], f32, tag="rec")
                        nc.vector.reciprocal_approx_fast(rec, pss)
                        ot = work.tile([128, QT], bf16, tag="ot")
                        nc.vector.scalar_tensor_tensor(
                            ot, pso, gbc[:, h:h + 1], rec, op0=MUL, op1=MUL)
                        half, hoff = qt // 2, (qt % 2) * QT
                        nc.sync.dma_start(
                            ag_in[half][h * 128:(h + 1) * 128, hoff:hoff + QT],
                            ot)
                    # issue the gather as soon as a sequence half completes
                    half = qt // 2
                    done_halves.add(qt)
                    if (half * 2 in done_halves) and (half * 2 + 1 in done_halves):
                        nc.gpsimd.collective_compute(
                            "AllGather", mybir.AluOpType.bypass,
                            replica_groups=[list(range(NC))],
                            ins=[ag_in[half][:].opt()],
                            outs=[ag_out[half][:].opt()])
                ag_outs.append(ag_out)

            # ================= o_proj, chunks in readiness order ==========
            lazy(wo_sb, wo_t, "wo")
            sched = [(b, rc) for b in range(B) for rc in range(S // OC)]
            for (b, rc) in sched:
                r0 = b * S
                g0 = rc * OC
                half, hoff = g0 // (S // 2), g0 % (S // 2)
                ag3 = ag_outs[b][half][:].rearrange("(ko p) r -> p ko r", p=128)
                gt = stream.tile([128, KO, OC], bf16, tag="stream")
                for kq in range(4):
                    nc.sync.dma_start(gt[:, kq * 4:(kq + 1) * 4],
                                      ag3[:, kq * 4:(kq + 1) * 4, hoff:hoff + OC])
                for ct in range(HDC // 128):
                    pso2 = psB.tile([128, QT], f32, tag="pv",
                                    name="pso2")[:, :OC]
                    for ko in range(KO):
                        nc.tensor.matmul(
                            pso2, lhsT=wo_sb[:, ko, ct * 128:(ct + 1) * 128],
                            rhs=gt[:, ko],
                            start=(ko == 0), stop=(ko == KO - 1))
                    oc_sb = work.tile([128, OC], f32, tag="oc")
                    nc.scalar.activation(oc_sb, pso2, AF.Copy)
                    nc.sync.dma_start(
                        out[ct * 128:(ct + 1) * 128, r0 + g0:r0 + g0 + OC],
                        oc_sb)
    nc.compile()
    return nc


def _prepare_in_maps(hidden_states, position_ids, Wq, Wk, Wv, Wo, Wg, bg):
    import ml_dtypes
    b16 = ml_dtypes.bfloat16

    x = np.ascontiguousarray(hidden_states.reshape(ROWS, HID), dtype=np.float32)
    # [chunks, 128, KO, CH]: per-(chunk, partition) data contiguous, so
    # every DMA line is 16KB
    xt_t = np.ascontiguousarray(
        x.reshape(B * NCH, CH, KO, 128).transpose(0, 3, 2, 1)).astype(b16)

    def tile_w(WT):  # [HID, cols] -> [128, KO, cols]
        return np.ascontiguousarray(
            WT.reshape(KO, 128, WT.shape[1]).transpose(1, 0, 2)).astype(b16)

    WqT = Wq.T.astype(np.float32)
    WkT = Wk.T.astype(np.float32)
    WvT = Wv.T.astype(np.float32)
    WoT = Wo.T.astype(np.float32)
    WgT = Wg.T.astype(np.float32)

    inv_freq = 1.0 / (ROPE_BASE ** (np.arange(0, HD, 2, dtype=np.float32) / HD))
    freqs = np.arange(S, dtype=np.float32)[:, None] * inv_freq[None, :]
    emb = np.concatenate([freqs, freqs], axis=-1)          # [S, HD]
    cos_t = np.cos(emb).astype(np.float32)
    sin_t = np.sin(emb).astype(np.float32)
    pos = np.asarray(position_ids).astype(np.int64)
    cosT = np.ascontiguousarray(
        np.concatenate([cos_t[pos[b]] for b in range(B)], axis=0).T)
    sinT = np.ascontiguousarray(
        np.concatenate([sin_t[pos[b]] for b in range(B)], axis=0).T)
    sinT[:HD // 2] *= -1.0   # rotate-half sign folded into the table

    P = np.zeros((HD, HD), dtype=np.float32)
    half = HD // 2
    P[np.arange(half), np.arange(half) + half] = -1.0
    P[np.arange(half, HD), np.arange(half)] = 1.0
    pmatT = np.ascontiguousarray(P.T).astype(b16)

    tri = (np.arange(128)[None, :] >= np.arange(128)[:, None]).astype(b16)
    ones = np.ones((128, 128), dtype=b16)
    ident = np.eye(128, dtype=b16)
    bgc = np.asarray(bg, dtype=np.float32)

    in_maps = []
    for c in range(NC):
        s0 = c * HDC
        in_maps.append({
            "xt_t": xt_t,
            "wq_t": tile_w(np.ascontiguousarray(WqT[:, s0:s0 + HDC])),
            "wk_t": tile_w(np.ascontiguousarray(WkT[:, s0:s0 + HDC])),
            "wv_t": tile_w(np.ascontiguousarray(WvT[:, s0:s0 + HDC])),
            "wo_t": tile_w(np.ascontiguousarray(WoT[:, s0:s0 + HDC])),
            "wg_t": tile_w(np.ascontiguousarray(
                WgT[:, c * HPC:(c + 1) * HPC])),
            "bg": np.ascontiguousarray(bgc[c * HPC:(c + 1) * HPC, None]),
            "cosT": cosT, "sinT": sinT, "pmatT": pmatT,
            "tri": tri, "ones": ones, "ident": ident,
        })
    return in_maps


LAST_RESULT = None


def kernel(hidden_states, attention_mask, position_ids, Wq, Wk, Wv, Wo, Wg, bg):
    global LAST_RESULT
    _install_ntff_hook()
    from concourse.bass_utils import run_bass_kernel_spmd

    if "nc" not in _CACHE:
        _CACHE["nc"] = _build()
    nc = _CACHE["nc"]

    in_maps = _prepare_in_maps(hidden_states, position_ids, Wq, Wk, Wv, Wo, Wg, bg)
    res = run_bass_kernel_spmd(nc, in_maps, core_ids=list(range(NC)))
    LAST_RESULT = res
    blocks = [res.results[c]["out"] for c in range(NC)]     # each [HDC, ROWS]
    full_T = np.concatenate(blocks, axis=0)                 # [HID, ROWS]
    return np.ascontiguousarray(full_T.T).reshape(B, S, HID).astype(np.float32)

